# revision 16
# baseline (speedup 1.0000x reference)
"""MeshCNN unpool kernel for 8 Trainium2 NeuronCores.

Computes out[b,f,t] = sum_e features[b,f,e] * unroll_mat[b,e,t] / occurrences[b,t]
(B=4, F=128, E=3000, T=6000), i.e. a batched matmul (B,F,E)@(B,E,T) with a
per-column normalization.

Sharding: 8 cores = (batch b, T-half) pairs. Each core streams its
[3000, 3000] unroll slice from HBM through the TensorEngine, accumulating
out = featT.T @ unroll in PSUM over e-tiles of 125 rows, then scales by
1/occurrences.

Precision scheme: unroll_mat is binary (0/1), so it is stored as fp8e4m3
losslessly (4x less HBM traffic than fp32; fp32 matmul would also run at
1/4 PE rate). The host also pre-swizzles/pads the per-core unroll slice into
the exact SBUF tile layout so every group DMA is one fully contiguous read.
features are decomposed host-side into a cascade of planes whose matmul
passes sum to the fp32 result in fp32 PSUM:
  - "bf16" plane: normal matmul, 24 e-tiles/plane (1 MAC/cell/cycle)
  - "f8dr" plane: fp8 DoubleRow matmul, 12 e-tile-pairs/plane
    (2 MAC/cell/cycle -> half the PE time of a bf16 plane)
All planes are scaled by 2^9 so fp8 residual planes sit in e4m3's normal
range; the scale is folded into the 1/occurrences reciprocal exactly.

SCHEME picks the plane cascade:
  "bf16x2"    hi+lo bf16 planes: ~2^-17 feature error, slowest
  "bf16+f8dr" bf16 hi + fp8 residual: ~2^-13 error, ~1.5x faster
  "f8drx2"    two fp8 planes: ~2^-8 error, fastest (PE), DMA-bound
"""

import numpy as np
import ml_dtypes

import concourse.mybir as mybir
import concourse.tile as tile
from concourse import bacc
from concourse.bass_utils import run_bass_kernel_spmd

B, F, E, T = 4, 128, 3000, 6000
N_CORES = 8
T_CORE = T // 2            # 3000 output columns per core
T_PAD = 3008               # padded SBUF row pitch (DoubleRow step%16==0)
EP = 125                   # e-tile partition size (3000 = 24 * 125)
NE = E // EP               # 24 e-tiles
A = 4                      # e-tiles per DMA group (even, for DR pairs)
NG = NE // A               # DMA groups
NBUF = 4                   # unroll tile buffers
CHUNK = 500                # psum free-dim chunk (<=512 fp32 / bank)
NCH = T_CORE // CHUNK      # 6 psum chunks -> 6 banks
SCHEME = "bf16+f8dr"
SCALE = 512.0              # 2^9 plane scale (exact in bf16/f32)

F32 = mybir.dt.float32
BF16 = mybir.dt.bfloat16
F8 = mybir.dt.float8e4
BF16_NP = ml_dtypes.bfloat16
F8_NP = ml_dtypes.float8_e4m3

PASS_KINDS = {
    "bf16x2": ("bf16", "bf16"),
    "bf16+f8dr": ("bf16", "f8dr"),
    "f8drx2": ("f8dr", "f8dr"),
}[SCHEME]


def build_kernel(repeat: int = 1, scheme: str = SCHEME, a: int = A,
                 nbuf: int = NBUF, out_split: bool = True,
                 chunk_major: bool = False, phases: int = 1):
    pass_kinds = {
        "bf16x2": ("bf16", "bf16"),
        "bf16+f8dr": ("bf16", "f8dr"),
        "f8drx2": ("f8dr", "f8dr"),
        "bf16x1": ("bf16",),
    }[scheme]
    ng = NE // a
    nc = bacc.Bacc("TRN2", target_bir_lowering=False, debug=False,
                   num_devices=N_CORES)

    plane_d = []
    for i, kind in enumerate(pass_kinds):
        dt = BF16 if kind == "bf16" else F8
        plane_d.append(nc.dram_tensor(f"plane{i}", [E, F], dt,
                                      kind="ExternalInput"))
    unroll_d = nc.dram_tensor("unroll", [ng * EP, a * T_PAD], F8,
                              kind="ExternalInput")
    occ_d = nc.dram_tensor("occ", [F, T_CORE], BF16, kind="ExternalInput")
    out_d = nc.dram_tensor("out", [F, T_CORE], F32, kind="ExternalOutput")

    # DRAM view of unroll (host pre-swizzled AND pre-padded to the SBUF
    # tile layout): group g -> [EP, a*T_PAD], fully contiguous both sides
    u_view = unroll_d[:].rearrange("(g p) x -> g p x", p=EP)

    with tile.TileContext(nc) as tc:
        with (
            tc.tile_pool(name="const", bufs=1) as const_pool,
            tc.tile_pool(name="upool", bufs=nbuf) as upool,
            tc.tile_pool(name="acc", bufs=1, space="PSUM") as acc_pool,
            tc.tile_pool(name="outp", bufs=2) as out_pool,
        ):
            # feature planes, host-pretransposed to [E, F]
            planes_sb = []
            for i, kind in enumerate(pass_kinds):
                if kind == "bf16":
                    p_sb = const_pool.tile([EP, NE, F], BF16, name=f"pl{i}")
                    nc.scalar.dma_start(
                        p_sb[:], plane_d[i][:].rearrange("(g p) f -> p g f",
                                                         p=EP))
                else:
                    p_sb = const_pool.tile([EP, NE // 2, 2, F], F8,
                                           name=f"pl{i}")
                    nc.scalar.dma_start(
                        p_sb[:],
                        plane_d[i][:].rearrange("(g two p) f -> p g two f",
                                                two=2, p=EP))
                planes_sb.append(p_sb)

            # 1/(SCALE*occ); host already broadcast occ and folded SCALE in
            # (SCALE*occ is a small int times 2^9 -> exact in bf16)
            occ_sb = const_pool.tile([F, T_CORE], BF16)
            nc.scalar.dma_start(occ_sb[:], occ_d[:])
            recip_sb = const_pool.tile([F, T_CORE], F32)
            nc.vector.reciprocal(recip_sb[:], occ_sb[:])

            n_mm_slots = ng * (a * pass_kinds.count("bf16")
                               + (a // 2) * pass_kinds.count("f8dr"))

            assert NCH % phases == 0
            nch_p = NCH // phases
            for _ in range(repeat):
                psums = [acc_pool.tile([F, CHUNK], F32, tag=f"ps{c}",
                                       name=f"ps{c}")
                         for c in range(NCH)]
                u_tiles = {}
                out_sb = out_pool.tile([F, T_CORE], F32)
                for ph in range(phases):
                    chunks = range(ph * nch_p, (ph + 1) * nch_p)
                    slot = 0
                    n_ph_slots = n_mm_slots
                    for g in range(ng):
                        if ph == 0:
                            u_tile = upool.tile([EP, a, T_PAD], F8)
                            u_tiles[g] = u_tile
                            # split each group load across both HWDGE queues
                            # (concurrent queues: ~2x effective DMA rate)
                            flat = u_tile[:].rearrange("p a t -> p (a t)")
                            W = a * T_PAD // 2
                            for s in range(2):
                                eng = (nc.sync, nc.scalar)[(g + s) % 2]
                                eng.dma_start(
                                    flat[:, s * W:(s + 1) * W],
                                    u_view[g][:, s * W:(s + 1) * W])
                        else:
                            u_tile = u_tiles[g]
                        mms = []
                        for i, kind in enumerate(pass_kinds):
                            if kind == "bf16":
                                for j in range(a):
                                    e = g * a + j
                                    mms.append((planes_sb[i][:, e, :],
                                                u_tile[:, j, :], None))
                            else:
                                for q in range(a // 2):
                                    gp = g * (a // 2) + q
                                    mms.append((planes_sb[i][:, gp, :, :],
                                                u_tile[:, 2 * q:2 * q + 2, :],
                                                mybir.MatmulPerfMode.DoubleRow))
                        n_group_slots = len(mms)
                        base_slot = slot
                        if chunk_major:
                            order = [(c, w) for c in chunks
                                     for w in range(n_group_slots)]
                        else:
                            order = [(c, w) for w in range(n_group_slots)
                                     for c in chunks]
                        for c, w in order:
                            lhsT, rhs, pm = mms[w]
                            rhs_c = (rhs[:, c * CHUNK:(c + 1) * CHUNK]
                                     if pm is None else
                                     rhs[:, :, c * CHUNK:(c + 1) * CHUNK])
                            nc.tensor.matmul(
                                psums[c][:], lhsT, rhs_c,
                                start=(base_slot == 0 and w == 0),
                                stop=(base_slot + n_group_slots == n_ph_slots
                                      and w == n_group_slots - 1),
                                perf_mode=pm,
                            )
                        slot += n_group_slots
                    for c in chunks:
                        nc.vector.tensor_mul(
                            out_sb[:, c * CHUNK:(c + 1) * CHUNK],
                            psums[c][:],
                            recip_sb[:, c * CHUNK:(c + 1) * CHUNK],
                        )
                        if out_split:
                            eng = nc.sync if c % 2 == 0 else nc.scalar
                            eng.dma_start(out_d[:, c * CHUNK:(c + 1) * CHUNK],
                                          out_sb[:, c * CHUNK:(c + 1) * CHUNK])
                if not out_split:
                    nc.sync.dma_start(out_d[:], out_sb[:])

    nc.compile()
    return nc


_NC_CACHE = {}


def _get_nc(repeat: int = 1):
    if repeat not in _NC_CACHE:
        _NC_CACHE[repeat] = build_kernel(repeat)
    return _NC_CACHE[repeat]


def make_planes(feat_t, pass_kinds=PASS_KINDS):
    """Cascade-decompose [E,F] fp32 features (x SCALE) into planes."""
    resid = feat_t.astype(np.float32) * SCALE
    planes = []
    for kind in pass_kinds:
        np_dt = BF16_NP if kind == "bf16" else F8_NP
        p = resid.astype(np_dt)
        planes.append(p)
        resid = resid - p.astype(np.float32)
    return planes


def make_in_maps(features, unroll_mat, occurrences, scheme=SCHEME, a=A):
    pass_kinds = {
        "bf16x2": ("bf16", "bf16"),
        "bf16+f8dr": ("bf16", "f8dr"),
        "f8drx2": ("f8dr", "f8dr"),
        "bf16x1": ("bf16",),
    }[scheme]
    ng = NE // a
    features = np.ascontiguousarray(features, dtype=np.float32)
    occurrences = np.ascontiguousarray(occurrences, dtype=np.float32)
    unroll8 = np.asarray(unroll_mat).astype(F8_NP)  # exact: values 0/1
    in_maps = []
    for c in range(N_CORES):
        b, h = divmod(c, 2)
        t0 = h * T_CORE
        occ_b = np.broadcast_to(
            (occurrences[b, t0:t0 + T_CORE] * SCALE).astype(BF16_NP),
            (F, T_CORE))
        u8 = np.zeros((ng, EP, a, T_PAD), dtype=F8_NP)
        u8[:, :, :, :T_CORE] = (unroll8[b, :, t0:t0 + T_CORE]
                                .reshape(ng, a, EP, T_CORE)
                                .transpose(0, 2, 1, 3))
        u8 = u8.reshape(ng * EP, a * T_PAD)
        m = {
            "unroll": np.ascontiguousarray(u8),
            "occ": np.ascontiguousarray(occ_b),
        }
        planes = make_planes(np.ascontiguousarray(features[b].T), pass_kinds)
        for i, p in enumerate(planes):
            m[f"plane{i}"] = p
        in_maps.append(m)
    return in_maps


def assemble(results):
    out = np.empty((B, F, T), dtype=np.float32)
    for c in range(N_CORES):
        b, h = divmod(c, 2)
        out[b, :, h * T_CORE:(h + 1) * T_CORE] = results[c]["out"]
    return out


def kernel(features, unroll_mat, occurrences):
    nc = _get_nc(repeat=1)
    in_maps = make_in_maps(features, unroll_mat, occurrences)
    res = run_bass_kernel_spmd(nc, in_maps, list(range(N_CORES)))
    return assemble(res.results)
